# revision 21
# baseline (speedup 1.0000x reference)
"""Trainium2 Bass kernel for nn_Model_29592324670139 (dense transformer).

Sharding: 8 cores = 4 pairs. Pair b handles batch item b; within a pair the
672-token sequence (21 vars x 32 windows, window-major order) is split by
window parity (rank0 = even windows, rank1 = odd windows), 336 tokens each.
Per layer, each core projects Q/K/V for its tokens; K/V are AllGathered
within the pair; attention/FFN/LN run on local tokens. The final pooled
feature sum is AllGathered, and head+MLP run redundantly per pair.

v2: fp8e4 DoubleRow(SwInterleave) matmuls for QKVO/FFN (weights x64),
batched softmax reciprocals, PE-matmul broadcasts, skew-absorbing dummy
collective, merged K+V AllGathers, deferred embedding scaling, f32r LN
stats.

Self-contained: hardcodes all shapes; only needs numpy/ml_dtypes/concourse.
"""

import numpy as np
import ml_dtypes

import concourse.bass as bass
import concourse.tile as tile
from concourse import bacc, mybir
from concourse.bass import ts, ds
from concourse.bass_utils import run_bass_kernel_spmd

F32 = mybir.dt.float32
F32R = mybir.dt.float32r
BF16 = mybir.dt.bfloat16
F8 = mybir.dt.float8e4
AX = mybir.AluOpType
AF = mybir.ActivationFunctionType
XL = mybir.AxisListType
PM = mybir.MatmulPerfMode

B, L, C = 4, 3072, 21
P, OUT, D, H, NL, DFF = 96, 96, 1024, 16, 2, 4096
NW = 32          # windows
SL = 336         # local tokens per core
S = 672          # full sequence
HD = 64          # head dim
NKC = D // 128   # 8 k-chunks of d_model
NKP = NKC // 2   # 4 k-chunk pairs (DoubleRow)
NFC = DFF // 128  # 32 chunks of d_ff
WS = 64.0        # fp8 weight scale
RW = 1.0 / WS

REPLICA_GROUPS = [[0, 1], [2, 3], [4, 5], [6, 7]]

# rank-invariant query-suffix starts per key chunk (block-causal skip)
SUF0 = [0, 105, 210, 0, 105, 210]

SWI = True       # DoubleRowSwInterleave for weight-stationary matmuls

KH = 128 * 4 * SL        # elems of half of K
VH = SL * 512            # elems of half of V
KVH = KH + VH

_BUILT = None
LAST_RESULT = None


# ----------------------------------------------------------------------------
# device program
# ----------------------------------------------------------------------------

def _build():
    nc = bacc.Bacc("TRN2", target_bir_lowering=False, debug=False,
                   enable_asserts=False, num_devices=8)

    t = {}

    def din(name, shape, dt):
        t[name] = nc.dram_tensor(name, list(shape), dt, kind="ExternalInput").ap()

    din("xfull", (C, L), F32)
    din("xloc", (P, SL), BF16)
    din("maskM", (112, 6, SL), F8)
    din("embW", (P, D), BF16)
    din("biases", (128, 2 * NKC + NL * (8 * NKC + NFC)), F32)
    din("bvpack", (1, NL * D), BF16)
    for l in range(NL):
        for w in ("Wq", "Wk"):
            if SWI:
                din(f"{w}{l}", (128, NKP, 2 * D), F8)    # SwI strips
            else:
                din(f"{w}{l}", (128, NKC, D), F8)        # plain (p, kc, n)
        din(f"Wo{l}", (128, NKC, D), BF16)
        din(f"Wv{l}", (2, 128, NKC, 512), F8)            # plain, per nh half
        if SWI:
            din(f"W1{l}", (4, 128, NKP, 2048), F8)       # SwI quarters
            din(f"W2{l}", (NKC, 128, NFC // 2, 256), F8)  # SwI per-oc
        else:
            din(f"W1{l}", (4, 128, NKC, 1024), F8)
            din(f"W2{l}", (NKC, 128, NFC, 128), F8)
    din("headWs", (128, NKC, OUT), BF16)
    din("hwsumN", (1, OUT), BF16)
    din("featC", (OUT, 1), F32)
    din("c1W", (OUT, 256), BF16)
    din("c1B", (128, 2), F32)
    din("c2W", (128, 2, 64), BF16)
    din("c2B", (64, 1), F32)
    din("c3W", (64, 2), BF16)
    din("c3B", (2, 1), F32)

    out_dram = nc.dram_tensor("out", [2, 1], F32, kind="ExternalOutput").ap()

    with tile.TileContext(nc) as tc:
        _emit(tc, t, out_dram)

    nc.compile()
    return nc, set(t.keys())


def _emit(tc, t, out_dram):
    from contextlib import ExitStack
    nc = tc.nc
    ctx = ExitStack()

    # ---------------- pools ----------------
    constp = ctx.enter_context(tc.tile_pool(name="constp", bufs=1))
    wpool = ctx.enter_context(tc.tile_pool(name="wpool", bufs=2))
    actp = ctx.enter_context(tc.tile_pool(name="actp", bufs=1))
    esbp = ctx.enter_context(tc.tile_pool(name="esbp", bufs=6))
    lnp = ctx.enter_context(tc.tile_pool(name="lnp", bufs=1))
    sqp = ctx.enter_context(tc.tile_pool(name="sqp", bufs=3))
    w2p = ctx.enter_context(tc.tile_pool(name="w2p", bufs=2))
    dramp = ctx.enter_context(tc.tile_pool(name="dramp", bufs=1, space="DRAM"))
    psp = ctx.enter_context(tc.tile_pool(name="psp", bufs=1, space="PSUM"))

    def ps_sc(name):
        return psp.tile([112, 2, 512], F32, name=name, tag="sc", bufs=3)

    def ps_sm(shape, name):
        return psp.tile(shape, F32, name=name, tag="sm", bufs=2,
                        padded_shape=[128, 512])

    def single(shape, dt, name, **kw):
        tl, free = tc.tile(shape, dt, name=name, **kw)
        ctx.callback(free)
        return tl

    # ---------------- skew-absorbing dummy collective ----------------
    dum_sb = constp.tile([1, 16], F32, name="dum_sb", tag="dum_sb")
    nc.vector.memset(dum_sb[:], 0.0)
    dum_in = dramp.tile([16], F32, name="dum_in", tag="dum_in")
    nc.sync.dma_start(out=dum_in[:], in_=dum_sb[:])
    dum_out = single([32], F32, "dum_out", space="DRAM", addr_space="Shared")
    nc.gpsimd.collective_compute(
        "AllGather", AX.bypass, replica_groups=REPLICA_GROUPS,
        ins=[dum_in[:]], outs=[dum_out[:]])


    NBC = 2 * NKC + NL * (8 * NKC + NFC)
    sb_bias = constp.tile([128, NBC], F32, name="sb_bias", tag="sb_bias")
    nc.gpsimd.dma_start(out=sb_bias[:], in_=t["biases"][:])
    _bc = [0]

    def bias_col(n=NKC):
        c0 = _bc[0]
        _bc[0] += n
        return sb_bias[:, c0:c0 + n]

    sb_embWsumN = bias_col()
    sb_embB = bias_col()
    bias_sb = {}
    for l in range(NL):
        for v in ("bq", "bk", "bo", "b2", "ln1s", "ln1b", "ln2s", "ln2b"):
            bias_sb[f"{v}{l}"] = bias_col()
        bias_sb[f"b1{l}"] = bias_col(NFC)

    # first-layer weights early (scalar DMA queue)
    def load_wsi(src, tag):
        shp = [128, NKP, 2 * D] if SWI else [128, NKC, D]
        w = wpool.tile(shp, F8, name="w_si", tag=tag, bufs=1)
        nc.scalar.dma_start(out=w[:], in_=src)
        return w

    def load_wv(l):
        wv = [wpool.tile([128, NKC, 512], F8, name="wv_t", tag=f"wv{i}",
                         bufs=1)
              for i in range(2)]
        for i in range(2):
            nc.scalar.dma_start(out=wv[i][:], in_=t[f"Wv{l}"][i])
        return wv

    wk = load_wsi(t["Wk0"][:], "wk")
    wv = load_wv(0)
    wq = load_wsi(t["Wq0"][:], "wq")

    # ---------------- hot-path loads ----------------
    xloc_sb = constp.tile([P, SL], BF16, name="xloc_sb", tag="xloc_sb")
    nc.sync.dma_start(out=xloc_sb[:], in_=t["xloc"][:])
    embW_sb = constp.tile([P, D], BF16, name="embW_sb", tag="embW_sb")
    nc.sync.dma_start(out=embW_sb[:], in_=t["embW"][:])
    # ---------------- stage 0: instance norm stats ----------------
    st6 = constp.tile([C, 6, 6], F32, name="st6", tag="st6")
    xfp = ctx.enter_context(tc.tile_pool(name="xfp", bufs=2))
    for i in range(6):
        xfc = xfp.tile([C, 512], F32, name="xfc", tag="xfc")
        nc.sync.dma_start(out=xfc[:], in_=t["xfull"][:, ts(i, 512)])
        nc.vector.bn_stats(out=st6[:, i, :], in_=xfc[:])
    mv = constp.tile([C, 2], F32, name="mv", tag="mv")
    nc.vector.bn_aggr(out=mv[:], in_=st6[:])
    eps6_sb = constp.tile([C, 1], F32, name="eps6_sb", tag="eps6_sb")
    nc.vector.memset(eps6_sb[:], 1e-6)
    eps5_sb = constp.tile([1, 1], F32, name="eps5_sb", tag="eps5_sb")
    nc.vector.memset(eps5_sb[:], 1e-5)
    std21 = constp.tile([C, 1], F32, name="std21", tag="std21")
    nc.scalar.activation(out=std21[:], in_=mv[:, 1:2], func=AF.Sqrt, bias=eps6_sb[:])
    stat2 = constp.tile([C, 2], F32, name="stat2", tag="stat2")
    nc.vector.reciprocal(out=stat2[:, 0:1], in_=std21[:])
    nc.vector.tensor_mul(stat2[:, 1:2], mv[:, 0:1], stat2[:, 0:1])

    stat_dram = dramp.tile([C, 2], F32, name="stat_dram", tag="stat_dram")
    nc.sync.dma_start(out=stat_dram[:], in_=stat2[:])
    rstd_tok = constp.tile([1, SL], F32, name="rstd_tok", tag="rstd_tok")
    nc.sync.dma_start(
        out=rstd_tok[:].rearrange("p (n c) -> p n c", c=C),
        in_=bass.AP(tensor=stat_dram[:].tensor, offset=stat_dram[:].offset,
                    ap=[[0, 16], [2, C]]))
    mrs_tok = constp.tile([1, SL], F32, name="mrs_tok", tag="mrs_tok")
    nc.sync.dma_start(
        out=mrs_tok[:].rearrange("p (n c) -> p n c", c=C),
        in_=bass.AP(tensor=stat_dram[:].tensor, offset=stat_dram[:].offset + 1,
                    ap=[[0, 16], [2, C]]))
    rt_b = constp.tile([128, SL], F32, name="rt_b", tag="rt_b")
    nc.gpsimd.partition_broadcast(out_ap=rt_b[:], in_ap=rstd_tok[:])
    mrs_b = constp.tile([128, SL], F32, name="mrs_b", tag="mrs_b")
    nc.gpsimd.partition_broadcast(out_ap=mrs_b[:], in_ap=mrs_tok[:])

    # ---------------- small constants ----------------
    sb_bvp = constp.tile([1, NL * D], BF16, name="sb_bvp", tag="sb_bvp")
    nc.gpsimd.dma_start(out=sb_bvp[:], in_=t["bvpack"][:])
    sb_bv = {l: sb_bvp[:, ds(l * D, D)] for l in range(NL)}
    sb_featC = constp.tile([OUT, 1], F32, name="sb_featC", tag="sb_featC")
    nc.gpsimd.dma_start(out=sb_featC[:], in_=t["featC"][:])
    sb_hwsumN = constp.tile([1, OUT], BF16, name="sb_hwsumN", tag="sb_hwsumN")
    nc.gpsimd.dma_start(out=sb_hwsumN[:], in_=t["hwsumN"][:])
    sb_c1B = constp.tile([128, 2], F32, name="sb_c1B", tag="sb_c1B")
    nc.gpsimd.dma_start(out=sb_c1B[:], in_=t["c1B"][:])
    sb_c2B = constp.tile([64, 1], F32, name="sb_c2B", tag="sb_c2B")
    nc.gpsimd.dma_start(out=sb_c2B[:], in_=t["c2B"][:])
    sb_c3B = constp.tile([2, 1], F32, name="sb_c3B", tag="sb_c3B")
    nc.gpsimd.dma_start(out=sb_c3B[:], in_=t["c3B"][:])
    sb_c1W = constp.tile([OUT, 256], BF16, name="sb_c1W", tag="sb_c1W")
    nc.gpsimd.dma_start(out=sb_c1W[:], in_=t["c1W"][:])
    sb_c2W = constp.tile([128, 2, 64], BF16, name="sb_c2W", tag="sb_c2W")
    nc.gpsimd.dma_start(out=sb_c2W[:], in_=t["c2W"][:])
    sb_c3W = constp.tile([64, 2], BF16, name="sb_c3W", tag="sb_c3W")
    nc.gpsimd.dma_start(out=sb_c3W[:], in_=t["c3W"][:])
    sb_headW = constp.tile([128, NKC, OUT], BF16, name="sb_headW", tag="sb_headW")
    nc.gpsimd.dma_start(out=sb_headW[:], in_=t["headWs"][:])
    sb_mask = constp.tile([112, 6, SL], F8, name="sb_mask", tag="sb_mask")
    nc.gpsimd.dma_start(out=sb_mask[:], in_=t["maskM"][:])

    ones_row = constp.tile([1, 128], BF16, name="ones_row", tag="ones_row")
    nc.vector.memset(ones_row[:], 1.0)
    ones_bf = constp.tile([128, 1], BF16, name="ones_bf", tag="ones_bf")
    nc.vector.memset(ones_bf[:], 1.0)


    # ---------------- persistent activations ----------------
    h_f32 = single([128, NKC, SL], F32, "h_f32")
    h_f8 = single([128, NKC, SL], F8, "h_f8")
    d1 = single([128, NKC, SL], F32, "d1")

    # ---------------- stage 1: embedding (raw mm first, scale later) -------
    for c8 in range(NKC):
        pse = ps_sm([128, SL], "pse")
        nc.tensor.matmul(pse[:], lhsT=embW_sb[:, ts(c8, 128)], rhs=xloc_sb[:],
                         start=True, stop=True)
        nc.scalar.activation(out=d1[:, c8, :], in_=pse[:], func=AF.Copy)
    for c8 in range(NKC):
        nc.vector.tensor_mul(d1[:, c8, :], d1[:, c8, :], rt_b[:])
        nc.vector.scalar_tensor_tensor(
            out=d1[:, c8, :], in0=mrs_b[:], scalar=sb_embWsumN[:, c8:c8 + 1],
            in1=d1[:, c8, :], op0=AX.mult, op1=AX.add)
        nc.scalar.activation(out=h_f32[:, c8, :], in_=d1[:, c8, :],
                             func=AF.Identity, bias=sb_embB[:, c8:c8 + 1])
        nc.scalar.activation(out=h_f8[:, c8, :], in_=d1[:, c8, :],
                             func=AF.Identity, bias=sb_embB[:, c8:c8 + 1])

    # ---------------- helpers ----------------
    def mm_dr(out_ps, w_si, oc, rhs_src, nkp=NKP):
        """DoubleRow(SwI) accumulation over all k-pairs: out += W[:,:,oc].T @ rhs."""
        for kp in range(nkp):
            if SWI:
                lhsT = w_si[:, kp, ds(oc * 256, 256)].rearrange(
                    "p (two m) -> p two m", two=2)
            else:
                lhsT = w_si[:, ds(2 * kp, 2), ds(oc * 128, 128)]
            nc.tensor.matmul(
                out_ps[:], lhsT=lhsT,
                rhs=rhs_src[:, ds(2 * kp, 2), :],
                start=(kp == 0), stop=(kp == nkp - 1),
                perf_mode=PM.DoubleRowSwInterleave if SWI else PM.DoubleRow)

    def ln_stats(src):
        """Feature-major LN stats via bf16 shadow matmuls."""
        ps_sum = ps_sm([1, SL], "ps_sum")
        ps_sq = ps_sm([1, SL], "ps_sq")
        for c8 in range(NKC):
            sbf = sqp.tile([128, SL], BF16, name="sbf", tag="sbf")
            nc.scalar.activation(out=sbf[:], in_=src[:, c8, :], func=AF.Identity)
            sq_c = sqp.tile([128, SL], BF16, name="sq_c", tag="sq_c")
            nc.vector.tensor_mul(sq_c[:], src[:, c8, :], src[:, c8, :])
            nc.tensor.matmul(ps_sum[:], lhsT=ones_bf[:],
                             rhs=sbf[:],
                             start=(c8 == 0), stop=(c8 == NKC - 1))
            nc.tensor.matmul(ps_sq[:], lhsT=ones_bf[:],
                             rhs=sq_c[:],
                             start=(c8 == 0), stop=(c8 == NKC - 1))
        mean_bf = lnp.tile([1, SL], BF16, name="mean_bf", tag="mean_bf")
        nc.scalar.activation(out=mean_bf[:], in_=ps_sum[:], func=AF.Copy,
                             scale=1.0 / D)
        ms1 = lnp.tile([1, SL], F32, name="ms1", tag="ms1")
        nc.vector.tensor_mul(ms1[:], mean_bf[:], mean_bf[:])
        var1 = lnp.tile([1, SL], F32, name="var1", tag="var1")
        nc.vector.scalar_tensor_tensor(out=var1[:], in0=ps_sq[:], scalar=1.0 / D,
                                       in1=ms1[:], op0=AX.mult, op1=AX.subtract)
        std1 = lnp.tile([1, SL], F32, name="std1", tag="std1")
        nc.scalar.activation(out=std1[:], in_=var1[:], func=AF.Sqrt, bias=eps5_sb[:])
        rec_f = lnp.tile([1, SL], F32, name="rec_f", tag="rec_f")
        nc.vector.reciprocal_approx_fast(out=rec_f[:], in_=std1[:])
        rec_bf = lnp.tile([1, SL], BF16, name="rec_bf", tag="rec_bf")
        nc.gpsimd.tensor_copy(out=rec_bf[:], in_=rec_f[:])
        return mean_bf, rec_bf, rec_f

    def ln(s_sb, b_sb, src):
        """Feature-major layernorm of src (f32) -> h_f32 + h_f8."""
        mean_bf, rec_bf, _ = ln_stats(src)
        mb_ps = ps_sm([128, SL], "mb_ps")
        nc.tensor.matmul(mb_ps[:], lhsT=ones_row[:], rhs=mean_bf[:],
                         start=True, stop=True)
        rb_ps = ps_sm([128, SL], "rb_ps")
        nc.tensor.matmul(rb_ps[:], lhsT=ones_row[:], rhs=rec_bf[:],
                         start=True, stop=True)
        for c8 in range(NKC):
            nc.vector.tensor_sub(d1[:, c8, :], src[:, c8, :], mb_ps[:])
            nc.vector.scalar_tensor_tensor(
                out=d1[:, c8, :], in0=d1[:, c8, :], scalar=s_sb[:, c8:c8 + 1],
                in1=rb_ps[:], op0=AX.mult, op1=AX.mult)
            nc.scalar.activation(out=h_f32[:, c8, :], in_=d1[:, c8, :],
                                 func=AF.Identity, bias=b_sb[:, c8:c8 + 1])
            nc.scalar.activation(out=h_f8[:, c8, :], in_=d1[:, c8, :],
                                 func=AF.Identity, bias=b_sb[:, c8:c8 + 1])

    # ---------------- transformer layers ----------------
    wo = None
    for l in range(NL):
        # K proj halves + V proj halves, merged bounce + AllGather per half
        k_sb = actp.tile([128, NKC, SL], BF16, name="k_sb", tag="k_sb")
        v_sb = actp.tile([112, 3, D], BF16, name="v_sb", tag="v_sb")
        bv_b = actp.tile([112, D], BF16, name="bv_b", tag="bv_b")
        nc.gpsimd.partition_broadcast(out_ap=bv_b[:], in_ap=sb_bv[l])
        bkc = bias_sb[f"bk{l}"]
        kvbnc_in = [dramp.tile([KVH], BF16, name=f"kvb_in{l}_{i}",
                               tag=f"kvb_in{l}_{i}") for i in range(2)]
        kvbnc_out = [single([2 * KVH], BF16, f"kvb_out{l}_{i}", space="DRAM",
                            addr_space="Shared") for i in range(2)]
        for i in range(2):
            for oc in range(4 * i, 4 * i + 4):
                psk = ps_sm([128, SL], "psk")
                mm_dr(psk, wk, oc, h_f8)
                nc.vector.tensor_scalar(
                    out=k_sb[:, oc, :], in0=psk[:], scalar1=bkc[:, oc:oc + 1],
                    scalar2=RW, op0=AX.add, op1=AX.mult)
            for tc3 in range(3):
                psv = ps_sm([112, 512], "psv")
                for kp in range(NKP):
                    nc.tensor.matmul(
                        psv[:], lhsT=h_f8[:, ds(2 * kp, 2), ts(tc3, 112)],
                        rhs=wv[i][:, ds(2 * kp, 2), :],
                        start=(kp == 0), stop=(kp == NKP - 1),
                        perf_mode=PM.DoubleRow)
                nc.vector.scalar_tensor_tensor(
                    out=v_sb[:, tc3, ts(i, 512)], in0=psv[:], scalar=RW,
                    in1=bv_b[:, ts(i, 512)], op0=AX.mult, op1=AX.add)
            nc.sync.dma_start(
                out=kvbnc_in[i][ds(0, KH)].rearrange("(kc p tk) -> p kc tk",
                                                     p=128, tk=SL),
                in_=k_sb[:, ds(i * 4, 4), :])
            nc.sync.dma_start(
                out=kvbnc_in[i][ds(KH, VH)].rearrange("(t3 p he) -> p t3 he",
                                                      p=112, he=512),
                in_=v_sb[:, :, ds(i * 512, 512)])
            nc.gpsimd.collective_compute(
                "AllGather", AX.bypass, replica_groups=REPLICA_GROUPS,
                ins=[kvbnc_in[i][:]], outs=[kvbnc_out[i][:]])

        # Q proj (overlaps the AllGathers)
        q_sb = actp.tile([128, NKC, SL], BF16, name="q_sb", tag="q_sb")
        bqc = bias_sb[f"bq{l}"]
        for oc in range(NKC):
            psq = ps_sm([128, SL], "psq")
            mm_dr(psq, wq, oc, h_f8)
            nc.vector.tensor_scalar(
                out=q_sb[:, oc, :], in0=psq[:], scalar1=bqc[:, oc:oc + 1],
                scalar2=RW, op0=AX.add, op1=AX.mult)

        # prefetch out-proj weights (bf16)
        wo = wpool.tile([128, NKC, D], BF16, name="wo_t", tag="wo", bufs=1)
        nc.scalar.dma_start(out=wo[:], in_=t[f"Wo{l}"][:])

        k_full = actp.tile([128, NKC, S], BF16, name="k_full", tag="k_full")
        v_full = actp.tile([112, 6, H, HD + 1], BF16, name="v_full", tag="v_full")
        nc.vector.memset(v_full[:, :, :, HD:HD + 1], 1.0)
        for i in range(2):
            for r in range(2):
                nc.scalar.dma_start(
                    out=k_full[:, ds(i * 4, 4), ds(r * SL, SL)],
                    in_=kvbnc_out[i][ds(r * KVH, KH)].rearrange(
                        "(kc p tk) -> p kc tk", p=128, tk=SL))
            for c6 in range(6):
                off = (c6 // 3) * KVH + KH + (c6 % 3) * 112 * 512
                nc.scalar.dma_start(
                    out=v_full[:, c6, ds(i * 8, 8), 0:HD],
                    in_=kvbnc_out[i][ds(off, 112 * 512)].rearrange(
                        "(p hh e) -> p hh e", p=112, e=HD))

        # ---------------- attention ----------------
        boc = bias_sb[f"bo{l}"]
        att_sb = actp.tile([128, NKC, SL], BF16, name="att_sb", tag="att_sb")
        stage_o = actp.tile([64, NKC, SL], BF16, name="stage_o", tag="stage_o")

        for hc in range(8):
            esb2s = []
            for cc in range(6):
                tqs = SUF0[cc]
                suf = SL - tqs
                ps2 = ps_sc("pss")
                for par in range(2):
                    nc.tensor.matmul(
                        ps2[:, par, 0:suf],
                        lhsT=k_full[ds(64 * par, 64), hc, ts(cc, 112)],
                        rhs=q_sb[ds(64 * par, 64), hc, ds(tqs, suf)],
                        start=True, stop=True)
                esb2 = esbp.tile([112, 2, SL], BF16, name="esb2", tag="esb")
                nc.scalar.activation(out=esb2[:, :, 0:suf],
                                     in_=ps2[:, :, 0:suf], func=AF.Exp)
                for par in range(2):
                    nc.vector.tensor_mul(esb2[:, par, 0:suf],
                                         esb2[:, par, 0:suf],
                                         sb_mask[:, cc, ds(tqs, suf)])
                esb2s.append(esb2)
            for par in range(2):
                hh = 2 * hc + par
                psa = ps_sm([HD + 1, SL], "psa")
                for cc in range(6):
                    tqs = SUF0[cc]
                    suf = SL - tqs
                    nc.tensor.matmul(psa[:, ds(tqs, suf)],
                                     lhsT=v_full[:, cc, hh, :],
                                     rhs=esb2s[cc][:, par, 0:suf],
                                     start=(cc == 0), stop=(cc == 5))
                den = lnp.tile([1, SL], F32, name="den", tag="den", bufs=2)
                nc.vector.tensor_copy(out=den[:], in_=psa[ds(HD, 1), :])
                rec = lnp.tile([1, SL], F32, name="rec", tag="rec", bufs=2)
                nc.vector.reciprocal_approx_fast(out=rec[:], in_=den[:])
                rb = lnp.tile([64, SL], F32, name="rb", tag="rb", bufs=2)
                nc.gpsimd.partition_broadcast(out_ap=rb[:], in_ap=rec[:])
                dst = att_sb[0:64, hc, :] if par == 0 else stage_o[:, hc, :]
                nc.vector.tensor_mul(dst, psa[0:HD, :], rb[:])
            if hc == 3 or hc == 7:
                half = hc // 4
                nc.sync.dma_start(out=att_sb[ds(64, 64), ds(half * 4, 4), :],
                                  in_=stage_o[:, ds(half * 4, 4), :])
                for oc in range(NKC):
                    pso = ps_sm([128, SL], "pso")
                    for kc in range(4 * half, 4 * half + 4):
                        nc.tensor.matmul(pso[:], lhsT=wo[:, kc, ts(oc, 128)],
                                         rhs=att_sb[:, kc, :],
                                         start=(kc == 4 * half),
                                         stop=(kc == 4 * half + 3))
                    if half == 0:
                        nc.vector.scalar_tensor_tensor(
                            out=d1[:, oc, :], in0=pso[:],
                            scalar=boc[:, oc:oc + 1],
                            in1=h_f32[:, oc, :], op0=AX.add, op1=AX.add)
                    else:
                        nc.vector.tensor_add(d1[:, oc, :], d1[:, oc, :],
                                             pso[:])
        ln(bias_sb[f"ln1s{l}"], bias_sb[f"ln1b{l}"], src=d1)

        # FFN
        g_f8 = actp.tile([128, NFC, SL], F8, name="g_f8", tag="g_f8")
        b1c = bias_sb[f"b1{l}"]
        for quad in range(4):
            w1 = wpool.tile([128, NKP, 2048] if SWI else [128, NKC, 1024],
                            F8, name="w1q", tag="w1q")
            nc.scalar.dma_start(out=w1[:], in_=t[f"W1{l}"][quad])
            for f8c in range(8):
                fc = quad * 8 + f8c
                psf = ps_sm([128, SL], "psf")
                mm_dr(psf, w1, f8c, h_f8)
                nc.scalar.activation(out=g_f8[:, fc, :], in_=psf[:],
                                     func=AF.Gelu, scale=RW,
                                     bias=b1c[:, fc:fc + 1])
        b2c = bias_sb[f"b2{l}"]
        for oc in range(NKC):
            w2 = w2p.tile([128, NFC // 2, 256] if SWI else [128, NFC, 128],
                          F8, name="w2oc", tag="w2oc")
            nc.scalar.dma_start(out=w2[:], in_=t[f"W2{l}"][oc])
            psy = ps_sm([128, SL], "psy")
            mm_dr(psy, w2, 0, g_f8, nkp=NFC // 2)
            nc.vector.tensor_scalar(
                out=d1[:, oc, :], in0=psy[:], scalar1=b2c[:, oc:oc + 1],
                scalar2=RW, op0=AX.add, op1=AX.mult)
            nc.vector.tensor_add(d1[:, oc, :], d1[:, oc, :], h_f32[:, oc, :])

        ln(bias_sb[f"ln2s{l}"], bias_sb[f"ln2b{l}"], src=d1)

        if l + 1 < NL:
            wk = load_wsi(t[f"Wk{l + 1}"][:], "wk")
            wv = load_wv(l + 1)
            wq = load_wsi(t[f"Wq{l + 1}"][:], "wq")

    # ---------------- final: fused LN_f + pooling, head, MLP ----------------
    mean_bf, recf_bf, r1f = ln_stats(h_f32)
    mr1 = lnp.tile([1, SL], F32, name="mr1", tag="mr1")
    nc.vector.tensor_mul(mr1[:], mean_bf[:], r1f[:])
    csc = lnp.tile([1, 1], F32, name="csc", tag="csc")
    nc.vector.reduce_sum(out=csc[:], in_=mr1[:], axis=XL.X)
    rbf_ps = ps_sm([128, SL], "rbf_ps")
    nc.tensor.matmul(rbf_ps[:], lhsT=ones_row[:], rhs=recf_bf[:],
                     start=True, stop=True)
    hsum = constp.tile([128, NKC], F32, name="hsum", tag="hsum")
    for c8 in range(NKC):
        nc.vector.scalar_tensor_tensor(
            out=d1[:, c8, :], in0=h_f32[:, c8, :], scalar=1.0, in1=rbf_ps[:],
            op0=AX.mult, op1=AX.mult, accum_out=hsum[:, c8:c8 + 1])
    FD = D + 8
    fin_in = dramp.tile([FD], F32, name="fin_in", tag="fin_in")
    nc.sync.dma_start(out=fin_in[ds(0, D)].rearrange("(kc p) -> p kc", p=128),
                      in_=hsum[:])
    zpad = lnp.tile([1, 8], F32, name="zpad", tag="zpad")
    nc.vector.memset(zpad[:], 0.0)
    nc.sync.dma_start(out=fin_in[ds(D, 8)], in_=zpad[:])
    nc.sync.dma_start(out=fin_in[ds(D, 1)], in_=csc[:])
    fin_out = single([2 * FD], F32, "fin_out", space="DRAM",
                     addr_space="Shared")
    nc.gpsimd.collective_compute(
        "AllGather", AX.bypass, replica_groups=REPLICA_GROUPS,
        ins=[fin_in[:]], outs=[fin_out[:]])
    ffx = constp.tile([128, NKC, 2], F32, name="ffx", tag="ffx")
    for r in range(2):
        nc.sync.dma_start(
            out=ffx[:, :, r],
            in_=fin_out[ds(r * FD, D)].rearrange("(kc p) -> p kc", p=128))
    csc2 = constp.tile([1, 2], F32, name="csc2", tag="csc2")
    for r in range(2):
        nc.sync.dma_start(out=csc2[:, r:r + 1], in_=fin_out[ds(r * FD + D, 1)])
    cst_bf = constp.tile([1, 1], BF16, name="cst_bf", tag="cst_bf")
    nc.vector.tensor_add(cst_bf[:], csc2[:, 0:1], csc2[:, 1:2])
    hbar_bf = constp.tile([128, NKC], BF16, name="hbar_bf", tag="hbar_bf")
    nc.vector.tensor_add(hbar_bf[:], ffx[:, :, 0], ffx[:, :, 1])

    psh = ps_sm([OUT, 1], "psh")
    for kc in range(NKC):
        nc.tensor.matmul(psh[:], lhsT=sb_headW[:, kc, :],
                         rhs=hbar_bf[:, kc:kc + 1],
                         start=(kc == 0), stop=False)
    nc.tensor.matmul(psh[:], lhsT=sb_hwsumN[:], rhs=cst_bf[:],
                     start=False, stop=True)
    feat_bf = constp.tile([OUT, 1], BF16, name="feat_bf", tag="feat_bf")
    nc.scalar.activation(out=feat_bf[:], in_=psh[:], func=AF.Identity,
                         bias=sb_featC[:])

    z1_bf = constp.tile([128, 2], BF16, name="z1_bf", tag="z1_bf")
    for i2 in range(2):
        psc = ps_sm([128, 1], "psc")
        nc.tensor.matmul(psc[:], lhsT=sb_c1W[:, ts(i2, 128)], rhs=feat_bf[:],
                         start=True, stop=True)
        nc.scalar.activation(out=z1_bf[:, i2:i2 + 1], in_=psc[:], func=AF.Relu,
                             bias=sb_c1B[:, i2:i2 + 1])
    psc2 = ps_sm([64, 1], "psc2")
    for kc in range(2):
        nc.tensor.matmul(psc2[:], lhsT=sb_c2W[:, kc, :], rhs=z1_bf[:, kc:kc + 1],
                         start=(kc == 0), stop=(kc == 1))
    z2_bf = constp.tile([64, 1], BF16, name="z2_bf", tag="z2_bf")
    nc.scalar.activation(out=z2_bf[:], in_=psc2[:], func=AF.Relu, bias=sb_c2B[:])
    psc3 = ps_sm([2, 1], "psc3")
    nc.tensor.matmul(psc3[:], lhsT=sb_c3W[:], rhs=z2_bf[:], start=True, stop=True)
    out_sb = constp.tile([2, 1], F32, name="out_sb", tag="out_sb")
    nc.scalar.activation(out=out_sb[:], in_=psc3[:], func=AF.Identity,
                         bias=sb_c3B[:])
    nc.sync.dma_start(out=out_dram[:], in_=out_sb[:])
    ctx.close()


# ----------------------------------------------------------------------------
# host side
# ----------------------------------------------------------------------------

def _bf16(x):
    return np.ascontiguousarray(np.asarray(x, dtype=np.float32)).astype(
        ml_dtypes.bfloat16)


def _f32(x):
    return np.ascontiguousarray(np.asarray(x, dtype=np.float32))


def _f8(x):
    return np.ascontiguousarray(
        np.clip(np.asarray(x, dtype=np.float32), -240.0, 240.0)).astype(
        ml_dtypes.float8_e4m3)


def _w8i(a, scale=WS):
    """[D_in, N] -> SwInterleave strips [128, NKP, 2N]: per k-pair, columns
    of a 128-col chunk reversed + A/B interleaved, chunks concatenated."""
    a = np.asarray(a, np.float32) * scale
    din, n = a.shape
    kc = din // 128
    ar = a.reshape(kc, 128, n // 128, 128)
    A = ar[0::2][..., ::-1]                     # [kc/2, 128, nc, 128] reversed
    Bm = ar[1::2][..., ::-1]
    inter = np.stack([A, Bm], axis=-1)          # [kc/2, 128, nc, 128, 2]
    inter = inter.transpose(1, 0, 2, 3, 4).reshape(128, kc // 2, 2 * n)
    return _f8(inter)


def _w8p(a, scale=WS):
    """[D_in, N] -> plain [128, D_in//128, N] fp8 (p, kc, n)."""
    a = np.asarray(a, np.float32) * scale
    din, n = a.shape
    return _f8(a.reshape(din // 128, 128, n).transpose(1, 0, 2))


def _wtile(a):
    a = np.asarray(a, np.float32)
    din, n = a.shape
    return _bf16(a.reshape(din // 128, 128, n).transpose(1, 0, 2))


def _w2comp(inp, l):
    """Static mean-compensation for fp8 W2: (W2 - W2q)^T @ E[gelu] where the
    gelu input is ~N(0, sigma_j) with sigma_j = ||diag(ln1_s) W1[:, j]||."""
    w2 = np.asarray(inp["W2"][l], np.float32)
    w2q = np.clip(w2 * WS, -240, 240).astype(ml_dtypes.float8_e4m3).astype(
        np.float32) / WS
    w1 = np.asarray(inp["W1"][l], np.float32)
    s = np.asarray(inp["ln1_s"][l], np.float32)
    sig = np.linalg.norm(w1 * s[:, None], axis=0)
    ebar = sig ** 2 / np.sqrt(2 * np.pi * (1 + sig ** 2))
    return (w2 - w2q).T @ ebar


def _btile(a, p=128):
    a = np.asarray(a, np.float32)
    return _f32(a.reshape(-1, p).T)


def _host_weights(inp):
    w = {}
    w["embW"] = _bf16(inp["emb_W"])
    bias_cols = [_btile(-np.asarray(inp["emb_W"], np.float32).sum(0)),
                 _btile(inp["emb_b"])]
    for l in range(NL):
        _wst = _w8i if SWI else _w8p
        w[f"Wq{l}"] = _wst(np.asarray(inp["Wq"][l], np.float32) * 0.125)
        w[f"Wk{l}"] = _wst(inp["Wk"][l])
        w[f"Wo{l}"] = _wtile(inp["Wo"][l])
        wv = _w8p(inp["Wv"][l])                  # [128, 8, 1024]
        w[f"Wv{l}"] = np.ascontiguousarray(
            wv.reshape(128, NKC, 2, 512).transpose(2, 0, 1, 3))
        w1 = np.asarray(inp["W1"][l], np.float32)
        if SWI:
            w1i = _w8i(w1)                       # [128, 4, 8192]
            w[f"W1{l}"] = np.ascontiguousarray(
                w1i.reshape(128, NKP, 4, 2048).transpose(2, 0, 1, 3))
        else:
            w1p = _w8p(w1)                       # [128, 8, 4096]
            w[f"W1{l}"] = np.ascontiguousarray(
                w1p.reshape(128, NKC, 4, 1024).transpose(2, 0, 1, 3))
        w2 = np.asarray(inp["W2"][l], np.float32)
        # per-oc SwI: treat each oc as [DFF, 128] -> [128part, NFC/2, 256]
        w2oc = []
        for oc in range(NKC):
            w2oc.append(_w8i(w2[:, ds_np(oc)]) if SWI
                        else _w8p(w2[:, ds_np(oc)]))
        w[f"W2{l}"] = np.ascontiguousarray(np.stack(w2oc))
        bias_cols += [
            _btile(np.asarray(inp["bq"][l], np.float32) * 8.0),
            _btile(np.asarray(inp["bk"][l], np.float32) * WS),
            _btile(inp["bo"][l]),
            _btile((np.asarray(inp["b2"][l], np.float32) + _w2comp(inp, l)) * WS),
            _btile(inp["ln1_s"][l]),
            _btile(inp["ln1_b"][l]),
            _btile(inp["ln2_s"][l]),
            _btile(inp["ln2_b"][l]),
            _btile(inp["b1"][l]),
        ]
    w["biases"] = _f32(np.concatenate(bias_cols, axis=1))
    w["bvpack"] = _bf16(np.concatenate(
        [np.asarray(inp["bv"][l], np.float32) for l in range(NL)])[None, :])
    lnfs = np.asarray(inp["lnf_s"], np.float32)
    lnfb = np.asarray(inp["lnf_b"], np.float32)
    hW = np.asarray(inp["head_W"], np.float32)
    hws = hW * lnfs[:, None] / S
    w["headWs"] = _wtile(hws)
    w["hwsumN"] = _bf16(-hws.sum(0)[None, :])
    w["featC"] = _f32((hW.T @ lnfb + np.asarray(inp["head_b"], np.float32))[:, None])
    w["c1W"] = _bf16(inp["c1_W"])
    w["c1B"] = _btile(inp["c1_b"])
    w["c2W"] = _wtile(inp["c2_W"])
    w["c2B"] = _f32(np.asarray(inp["c2_b"], np.float32)[:, None])
    w["c3W"] = _bf16(inp["c3_W"])
    w["c3B"] = _f32(np.asarray(inp["c3_b"], np.float32)[:, None])
    return w


def ds_np(oc):
    return slice(oc * 128, (oc + 1) * 128)


def kernel(**inputs):
    global _BUILT, LAST_RESULT
    if _BUILT is None:
        _BUILT = _build()
    nc, names = _BUILT

    w = _host_weights(inputs)
    x = np.asarray(inputs["x"], np.float32)

    wk = np.concatenate([np.repeat(np.arange(16) * 2, C),
                         np.repeat(np.arange(16) * 2 + 1, C)])
    in_maps = []
    for core in range(8):
        b, parity = core // 2, core % 2
        wins = np.arange(16) * 2 + parity
        xb = x[b]
        xl = np.empty((P, SL), np.float32)
        for i, wn in enumerate(wins):
            xl[:, i * C:(i + 1) * C] = xb[wn * P:(wn + 1) * P, :]
        wq = np.repeat(wins, C)
        mask = (wk[:, None] <= wq[None, :]).astype(np.float32)
        mask3 = mask.reshape(6, 112, SL).transpose(1, 0, 2)
        m = dict(w)
        m["xfull"] = _f32(xb.T)
        m["xloc"] = _bf16(xl)
        m["maskM"] = _f8(mask3)
        in_maps.append(m)

    res = run_bass_kernel_spmd(nc, in_maps, core_ids=list(range(8)))
    LAST_RESULT = res
    logits = np.stack(
        [res.results[2 * b]["out"].reshape(2).astype(np.float32) for b in range(B)])
    return logits


# revision 24
# speedup vs baseline: 1.0237x; 1.0237x over previous
"""Trainium2 Bass kernel for nn_Model_29592324670139 (dense transformer).

Sharding: 8 cores = 4 pairs. Pair b handles batch item b; within a pair the
672-token sequence (21 vars x 32 windows, window-major order) is split by
window parity (rank0 = even windows, rank1 = odd windows), 336 tokens each.
Per layer, each core projects Q/K/V for its tokens; K/V are AllGathered
within the pair; attention/FFN/LN run on local tokens. The final pooled
feature sum is AllGathered, and head+MLP run redundantly per pair.

v2: fp8e4 DoubleRow(SwInterleave) matmuls for QKVO/FFN (weights x64),
batched softmax reciprocals, PE-matmul broadcasts, skew-absorbing dummy
collective, merged K+V AllGathers, deferred embedding scaling, f32r LN
stats.

Self-contained: hardcodes all shapes; only needs numpy/ml_dtypes/concourse.
"""

import numpy as np
import ml_dtypes

import concourse.bass as bass
import concourse.tile as tile
from concourse import bacc, mybir
from concourse.bass import ts, ds
from concourse.bass_utils import run_bass_kernel_spmd

F32 = mybir.dt.float32
F32R = mybir.dt.float32r
BF16 = mybir.dt.bfloat16
F8 = mybir.dt.float8e4
AX = mybir.AluOpType
AF = mybir.ActivationFunctionType
XL = mybir.AxisListType
PM = mybir.MatmulPerfMode

B, L, C = 4, 3072, 21
P, OUT, D, H, NL, DFF = 96, 96, 1024, 16, 2, 4096
NW = 32          # windows
SL = 336         # local tokens per core
S = 672          # full sequence
HD = 64          # head dim
NKC = D // 128   # 8 k-chunks of d_model
NKP = NKC // 2   # 4 k-chunk pairs (DoubleRow)
NFC = DFF // 128  # 32 chunks of d_ff
WS = 64.0        # fp8 weight scale
RW = 1.0 / WS

REPLICA_GROUPS = [[0, 1], [2, 3], [4, 5], [6, 7]]

# rank-invariant query-suffix starts per key chunk (block-causal skip)
SUF0 = [0, 105, 210, 0, 105, 210]

SWI = True       # DoubleRowSwInterleave for weight-stationary matmuls

KH = 128 * 4 * SL        # elems of half of K
VH = SL * 512            # elems of half of V
KVH = KH + VH

_BUILT = None
LAST_RESULT = None


# ----------------------------------------------------------------------------
# device program
# ----------------------------------------------------------------------------

def _build():
    nc = bacc.Bacc("TRN2", target_bir_lowering=False, debug=False,
                   enable_asserts=False, num_devices=8)

    t = {}

    def din(name, shape, dt):
        t[name] = nc.dram_tensor(name, list(shape), dt, kind="ExternalInput").ap()

    din("xfull", (C, L), F32)
    din("xloc", (P, SL), BF16)
    din("maskM", (112, 6, SL), F8)
    din("embW", (P, D), BF16)
    din("biases", (128, 2 * NKC + NL * (8 * NKC + NFC)), F32)
    din("bvpack", (1, NL * D), BF16)
    for l in range(NL):
        for w in ("Wq", "Wk"):
            if SWI:
                din(f"{w}{l}", (128, NKP, 2 * D), F8)    # SwI strips
            else:
                din(f"{w}{l}", (128, NKC, D), F8)        # plain (p, kc, n)
        din(f"Wo{l}", (128, NKC, D), BF16)
        din(f"Wv{l}", (2, 128, NKC, 512), F8)            # plain, per nh half
        if SWI:
            din(f"W1{l}", (4, 128, NKP, 2048), F8)       # SwI quarters
            din(f"W2{l}", (NKC, 128, NFC // 2, 256), F8)  # SwI per-oc
        else:
            din(f"W1{l}", (4, 128, NKC, 1024), F8)
            din(f"W2{l}", (NKC, 128, NFC, 128), F8)
    din("headWs", (128, NKC, OUT), BF16)
    din("hwsumN", (1, OUT), BF16)
    din("featC", (OUT, 1), F32)
    din("c1W", (OUT, 256), BF16)
    din("c1B", (128, 2), F32)
    din("c2W", (128, 2, 64), BF16)
    din("c2B", (64, 1), F32)
    din("c3W", (64, 2), BF16)
    din("c3B", (2, 1), F32)

    out_dram = nc.dram_tensor("out", [2, 1], F32, kind="ExternalOutput").ap()

    with tile.TileContext(nc) as tc:
        _emit(tc, t, out_dram)

    nc.compile()
    return nc, set(t.keys())


def _emit(tc, t, out_dram):
    from contextlib import ExitStack
    nc = tc.nc
    ctx = ExitStack()

    # ---------------- pools ----------------
    constp = ctx.enter_context(tc.tile_pool(name="constp", bufs=1))
    wpool = ctx.enter_context(tc.tile_pool(name="wpool", bufs=2))
    actp = ctx.enter_context(tc.tile_pool(name="actp", bufs=1))
    esbp = ctx.enter_context(tc.tile_pool(name="esbp", bufs=8))
    lnp = ctx.enter_context(tc.tile_pool(name="lnp", bufs=1))
    sqp = ctx.enter_context(tc.tile_pool(name="sqp", bufs=3))
    w2p = ctx.enter_context(tc.tile_pool(name="w2p", bufs=2))
    dramp = ctx.enter_context(tc.tile_pool(name="dramp", bufs=1, space="DRAM"))
    psp = ctx.enter_context(tc.tile_pool(name="psp", bufs=1, space="PSUM"))

    def ps_sc(name):
        return psp.tile([112, 2, 512], F32, name=name, tag="sc", bufs=3)

    def ps_sm(shape, name):
        return psp.tile(shape, F32, name=name, tag="sm", bufs=2,
                        padded_shape=[128, 512])

    def single(shape, dt, name, **kw):
        tl, free = tc.tile(shape, dt, name=name, **kw)
        ctx.callback(free)
        return tl

    # ---------------- skew-absorbing dummy collective ----------------
    dum_sb = constp.tile([1, 16], F32, name="dum_sb", tag="dum_sb")
    nc.vector.memset(dum_sb[:], 0.0)
    dum_in = dramp.tile([16], F32, name="dum_in", tag="dum_in")
    nc.sync.dma_start(out=dum_in[:], in_=dum_sb[:])
    dum_out = single([32], F32, "dum_out", space="DRAM", addr_space="Shared")
    nc.gpsimd.collective_compute(
        "AllGather", AX.bypass, replica_groups=REPLICA_GROUPS,
        ins=[dum_in[:]], outs=[dum_out[:]])

    # ---------------- hot-path loads ----------------
    xloc_sb = constp.tile([P, SL], BF16, name="xloc_sb", tag="xloc_sb")
    nc.sync.dma_start(out=xloc_sb[:], in_=t["xloc"][:])
    embW_sb = constp.tile([P, D], BF16, name="embW_sb", tag="embW_sb")
    nc.sync.dma_start(out=embW_sb[:], in_=t["embW"][:])

    NBC = 2 * NKC + NL * (8 * NKC + NFC)
    sb_bias = constp.tile([128, NBC], F32, name="sb_bias", tag="sb_bias")
    nc.sync.dma_start(out=sb_bias[:], in_=t["biases"][:])
    _bc = [0]

    def bias_col(n=NKC):
        c0 = _bc[0]
        _bc[0] += n
        return sb_bias[:, c0:c0 + n]

    sb_embWsumN = bias_col()
    sb_embB = bias_col()
    bias_sb = {}
    for l in range(NL):
        for v in ("bq", "bk", "bo", "b2", "ln1s", "ln1b", "ln2s", "ln2b"):
            bias_sb[f"{v}{l}"] = bias_col()
        bias_sb[f"b1{l}"] = bias_col(NFC)

    # first-layer weights early (scalar DMA queue)
    def load_wsi(src, tag):
        shp = [128, NKP, 2 * D] if SWI else [128, NKC, D]
        w = wpool.tile(shp, F8, name="w_si", tag=tag, bufs=1)
        nc.scalar.dma_start(out=w[:], in_=src)
        return w

    def load_wv(l):
        wv = [wpool.tile([128, NKC, 512], F8, name="wv_t", tag=f"wv{i}",
                         bufs=1)
              for i in range(2)]
        for i in range(2):
            nc.scalar.dma_start(out=wv[i][:], in_=t[f"Wv{l}"][i])
        return wv

    wk = load_wsi(t["Wk0"][:], "wk")
    wv = load_wv(0)
    wq = load_wsi(t["Wq0"][:], "wq")

    # ---------------- stage 0: instance norm stats ----------------
    st6 = constp.tile([C, 6, 6], F32, name="st6", tag="st6")
    xfp = ctx.enter_context(tc.tile_pool(name="xfp", bufs=2))
    for i in range(6):
        xfc = xfp.tile([C, 512], F32, name="xfc", tag="xfc")
        nc.sync.dma_start(out=xfc[:], in_=t["xfull"][:, ts(i, 512)])
        nc.vector.bn_stats(out=st6[:, i, :], in_=xfc[:])
    mv = constp.tile([C, 2], F32, name="mv", tag="mv")
    nc.vector.bn_aggr(out=mv[:], in_=st6[:])
    eps6_sb = constp.tile([C, 1], F32, name="eps6_sb", tag="eps6_sb")
    nc.vector.memset(eps6_sb[:], 1e-6)
    eps5_sb = constp.tile([1, 1], F32, name="eps5_sb", tag="eps5_sb")
    nc.vector.memset(eps5_sb[:], 1e-5)
    std21 = constp.tile([C, 1], F32, name="std21", tag="std21")
    nc.scalar.activation(out=std21[:], in_=mv[:, 1:2], func=AF.Sqrt, bias=eps6_sb[:])
    stat2 = constp.tile([C, 2], F32, name="stat2", tag="stat2")
    nc.vector.reciprocal(out=stat2[:, 0:1], in_=std21[:])
    nc.vector.tensor_mul(stat2[:, 1:2], mv[:, 0:1], stat2[:, 0:1])

    stat_dram = dramp.tile([C, 2], F32, name="stat_dram", tag="stat_dram")
    nc.sync.dma_start(out=stat_dram[:], in_=stat2[:])
    rstd_tok = constp.tile([1, SL], F32, name="rstd_tok", tag="rstd_tok")
    nc.sync.dma_start(
        out=rstd_tok[:].rearrange("p (n c) -> p n c", c=C),
        in_=bass.AP(tensor=stat_dram[:].tensor, offset=stat_dram[:].offset,
                    ap=[[0, 16], [2, C]]))
    mrs_tok = constp.tile([1, SL], F32, name="mrs_tok", tag="mrs_tok")
    nc.sync.dma_start(
        out=mrs_tok[:].rearrange("p (n c) -> p n c", c=C),
        in_=bass.AP(tensor=stat_dram[:].tensor, offset=stat_dram[:].offset + 1,
                    ap=[[0, 16], [2, C]]))
    rt_b = constp.tile([128, SL], F32, name="rt_b", tag="rt_b")
    nc.gpsimd.partition_broadcast(out_ap=rt_b[:], in_ap=rstd_tok[:])
    mrs_b = constp.tile([128, SL], F32, name="mrs_b", tag="mrs_b")
    nc.gpsimd.partition_broadcast(out_ap=mrs_b[:], in_ap=mrs_tok[:])

    # ---------------- small constants ----------------
    sb_bvp = constp.tile([1, NL * D], BF16, name="sb_bvp", tag="sb_bvp")
    nc.gpsimd.dma_start(out=sb_bvp[:], in_=t["bvpack"][:])
    sb_bv = {l: sb_bvp[:, ds(l * D, D)] for l in range(NL)}
    sb_featC = constp.tile([OUT, 1], F32, name="sb_featC", tag="sb_featC")
    nc.gpsimd.dma_start(out=sb_featC[:], in_=t["featC"][:])
    sb_hwsumN = constp.tile([1, OUT], BF16, name="sb_hwsumN", tag="sb_hwsumN")
    nc.gpsimd.dma_start(out=sb_hwsumN[:], in_=t["hwsumN"][:])
    sb_c1B = constp.tile([128, 2], F32, name="sb_c1B", tag="sb_c1B")
    nc.gpsimd.dma_start(out=sb_c1B[:], in_=t["c1B"][:])
    sb_c2B = constp.tile([64, 1], F32, name="sb_c2B", tag="sb_c2B")
    nc.gpsimd.dma_start(out=sb_c2B[:], in_=t["c2B"][:])
    sb_c3B = constp.tile([2, 1], F32, name="sb_c3B", tag="sb_c3B")
    nc.gpsimd.dma_start(out=sb_c3B[:], in_=t["c3B"][:])
    sb_c1W = constp.tile([OUT, 256], BF16, name="sb_c1W", tag="sb_c1W")
    nc.gpsimd.dma_start(out=sb_c1W[:], in_=t["c1W"][:])
    sb_c2W = constp.tile([128, 2, 64], BF16, name="sb_c2W", tag="sb_c2W")
    nc.gpsimd.dma_start(out=sb_c2W[:], in_=t["c2W"][:])
    sb_c3W = constp.tile([64, 2], BF16, name="sb_c3W", tag="sb_c3W")
    nc.gpsimd.dma_start(out=sb_c3W[:], in_=t["c3W"][:])
    sb_headW = constp.tile([128, NKC, OUT], BF16, name="sb_headW", tag="sb_headW")
    nc.gpsimd.dma_start(out=sb_headW[:], in_=t["headWs"][:])
    sb_mask = constp.tile([112, 6, SL], F8, name="sb_mask", tag="sb_mask")
    nc.gpsimd.dma_start(out=sb_mask[:], in_=t["maskM"][:])

    ones_row = constp.tile([1, 128], BF16, name="ones_row", tag="ones_row")
    nc.vector.memset(ones_row[:], 1.0)
    ones_bf = constp.tile([128, 1], BF16, name="ones_bf", tag="ones_bf")
    nc.vector.memset(ones_bf[:], 1.0)


    # ---------------- persistent activations ----------------
    h_f32 = single([128, NKC, SL], F32, "h_f32")
    h_f8 = single([128, NKC, SL], F8, "h_f8")
    d1 = single([128, NKC, SL], F32, "d1")

    # ---------------- stage 1: embedding (raw mm first, scale later) -------
    for c8 in range(NKC):
        pse = ps_sm([128, SL], "pse")
        nc.tensor.matmul(pse[:], lhsT=embW_sb[:, ts(c8, 128)], rhs=xloc_sb[:],
                         start=True, stop=True)
        nc.scalar.activation(out=d1[:, c8, :], in_=pse[:], func=AF.Copy)
    for c8 in range(NKC):
        nc.vector.tensor_mul(d1[:, c8, :], d1[:, c8, :], rt_b[:])
        nc.vector.scalar_tensor_tensor(
            out=d1[:, c8, :], in0=mrs_b[:], scalar=sb_embWsumN[:, c8:c8 + 1],
            in1=d1[:, c8, :], op0=AX.mult, op1=AX.add)
        nc.scalar.activation(out=h_f32[:, c8, :], in_=d1[:, c8, :],
                             func=AF.Identity, bias=sb_embB[:, c8:c8 + 1])
        nc.scalar.activation(out=h_f8[:, c8, :], in_=d1[:, c8, :],
                             func=AF.Identity, bias=sb_embB[:, c8:c8 + 1])

    # ---------------- helpers ----------------
    def mm_dr(out_ps, w_si, oc, rhs_src, nkp=NKP):
        """DoubleRow(SwI) accumulation over all k-pairs: out += W[:,:,oc].T @ rhs."""
        for kp in range(nkp):
            if SWI:
                lhsT = w_si[:, kp, ds(oc * 256, 256)].rearrange(
                    "p (two m) -> p two m", two=2)
            else:
                lhsT = w_si[:, ds(2 * kp, 2), ds(oc * 128, 128)]
            nc.tensor.matmul(
                out_ps[:], lhsT=lhsT,
                rhs=rhs_src[:, ds(2 * kp, 2), :],
                start=(kp == 0), stop=(kp == nkp - 1),
                perf_mode=PM.DoubleRowSwInterleave if SWI else PM.DoubleRow)

    def ln_stats(src):
        """Feature-major LN stats via bf16 shadow matmuls."""
        ps_sum = ps_sm([1, SL], "ps_sum")
        ps_sq = ps_sm([1, SL], "ps_sq")
        for c8 in range(NKC):
            sbf = sqp.tile([128, SL], BF16, name="sbf", tag="sbf")
            nc.scalar.activation(out=sbf[:], in_=src[:, c8, :], func=AF.Identity)
            sq_c = sqp.tile([128, SL], BF16, name="sq_c", tag="sq_c")
            nc.vector.tensor_mul(sq_c[:], src[:, c8, :], src[:, c8, :])
            nc.tensor.matmul(ps_sum[:], lhsT=ones_bf[:],
                             rhs=sbf[:],
                             start=(c8 == 0), stop=(c8 == NKC - 1))
            nc.tensor.matmul(ps_sq[:], lhsT=ones_bf[:],
                             rhs=sq_c[:],
                             start=(c8 == 0), stop=(c8 == NKC - 1))
        mean_bf = lnp.tile([1, SL], BF16, name="mean_bf", tag="mean_bf")
        nc.scalar.activation(out=mean_bf[:], in_=ps_sum[:], func=AF.Copy,
                             scale=1.0 / D)
        ms1 = lnp.tile([1, SL], F32, name="ms1", tag="ms1")
        nc.vector.tensor_mul(ms1[:], mean_bf[:], mean_bf[:])
        var1 = lnp.tile([1, SL], F32, name="var1", tag="var1")
        nc.vector.scalar_tensor_tensor(out=var1[:], in0=ps_sq[:], scalar=1.0 / D,
                                       in1=ms1[:], op0=AX.mult, op1=AX.subtract)
        std1 = lnp.tile([1, SL], F32, name="std1", tag="std1")
        nc.scalar.activation(out=std1[:], in_=var1[:], func=AF.Sqrt, bias=eps5_sb[:])
        rec_f = lnp.tile([1, SL], F32, name="rec_f", tag="rec_f")
        nc.vector.reciprocal_approx_fast(out=rec_f[:], in_=std1[:])
        rec_bf = lnp.tile([1, SL], BF16, name="rec_bf", tag="rec_bf")
        nc.gpsimd.tensor_copy(out=rec_bf[:], in_=rec_f[:])
        return mean_bf, rec_bf, rec_f

    def ln(s_sb, b_sb, src):
        """Feature-major layernorm of src (f32) -> h_f32 + h_f8."""
        mean_bf, rec_bf, _ = ln_stats(src)
        mb_ps = ps_sm([128, SL], "mb_ps")
        nc.tensor.matmul(mb_ps[:], lhsT=ones_row[:], rhs=mean_bf[:],
                         start=True, stop=True)
        rb_ps = ps_sm([128, SL], "rb_ps")
        nc.tensor.matmul(rb_ps[:], lhsT=ones_row[:], rhs=rec_bf[:],
                         start=True, stop=True)
        for c8 in range(NKC):
            nc.vector.tensor_sub(d1[:, c8, :], src[:, c8, :], mb_ps[:])
            nc.vector.scalar_tensor_tensor(
                out=d1[:, c8, :], in0=d1[:, c8, :], scalar=s_sb[:, c8:c8 + 1],
                in1=rb_ps[:], op0=AX.mult, op1=AX.mult)
            nc.scalar.activation(out=h_f32[:, c8, :], in_=d1[:, c8, :],
                                 func=AF.Identity, bias=b_sb[:, c8:c8 + 1])
            nc.scalar.activation(out=h_f8[:, c8, :], in_=d1[:, c8, :],
                                 func=AF.Identity, bias=b_sb[:, c8:c8 + 1])

    # ---------------- transformer layers ----------------
    wo = None
    for l in range(NL):
        # K proj halves + V proj halves, merged bounce + AllGather per half
        k_sb = actp.tile([128, NKC, SL], BF16, name="k_sb", tag="k_sb")
        v_sb = actp.tile([112, 3, D], BF16, name="v_sb", tag="v_sb")
        bv_b = actp.tile([112, D], BF16, name="bv_b", tag="bv_b")
        nc.gpsimd.partition_broadcast(out_ap=bv_b[:], in_ap=sb_bv[l])
        bkc = bias_sb[f"bk{l}"]
        kvbnc_in = [dramp.tile([KVH], BF16, name=f"kvb_in{l}_{i}",
                               tag=f"kvb_in{l}_{i}") for i in range(2)]
        kvbnc_out = [single([2 * KVH], BF16, f"kvb_out{l}_{i}", space="DRAM",
                            addr_space="Shared") for i in range(2)]
        for i in range(2):
            for oc in range(4 * i, 4 * i + 4):
                psk = ps_sm([128, SL], "psk")
                mm_dr(psk, wk, oc, h_f8)
                nc.vector.tensor_scalar(
                    out=k_sb[:, oc, :], in0=psk[:], scalar1=bkc[:, oc:oc + 1],
                    scalar2=RW, op0=AX.add, op1=AX.mult)
            for tc3 in range(3):
                psv = ps_sm([112, 512], "psv")
                for kp in range(NKP):
                    nc.tensor.matmul(
                        psv[:], lhsT=h_f8[:, ds(2 * kp, 2), ts(tc3, 112)],
                        rhs=wv[i][:, ds(2 * kp, 2), :],
                        start=(kp == 0), stop=(kp == NKP - 1),
                        perf_mode=PM.DoubleRow)
                nc.vector.scalar_tensor_tensor(
                    out=v_sb[:, tc3, ts(i, 512)], in0=psv[:], scalar=RW,
                    in1=bv_b[:, ts(i, 512)], op0=AX.mult, op1=AX.add)
            nc.sync.dma_start(
                out=kvbnc_in[i][ds(0, KH)].rearrange("(kc p tk) -> p kc tk",
                                                     p=128, tk=SL),
                in_=k_sb[:, ds(i * 4, 4), :])
            nc.sync.dma_start(
                out=kvbnc_in[i][ds(KH, VH)].rearrange("(t3 p he) -> p t3 he",
                                                      p=112, he=512),
                in_=v_sb[:, :, ds(i * 512, 512)])
            nc.gpsimd.collective_compute(
                "AllGather", AX.bypass, replica_groups=REPLICA_GROUPS,
                ins=[kvbnc_in[i][:]], outs=[kvbnc_out[i][:]])

        # Q proj (overlaps the AllGathers)
        q_sb = actp.tile([128, NKC, SL], BF16, name="q_sb", tag="q_sb")
        bqc = bias_sb[f"bq{l}"]
        for oc in range(NKC):
            psq = ps_sm([128, SL], "psq")
            mm_dr(psq, wq, oc, h_f8)
            nc.vector.tensor_scalar(
                out=q_sb[:, oc, :], in0=psq[:], scalar1=bqc[:, oc:oc + 1],
                scalar2=RW, op0=AX.add, op1=AX.mult)

        # prefetch out-proj weights (bf16)
        wo = wpool.tile([128, NKC, D], BF16, name="wo_t", tag="wo", bufs=1)
        nc.scalar.dma_start(out=wo[:], in_=t[f"Wo{l}"][:])

        k_full = actp.tile([128, NKC, S], BF16, name="k_full", tag="k_full")
        for i in range(2):
            for r in range(2):
                nc.sync.dma_start(
                    out=k_full[:, ds(i * 4, 4), ds(r * SL, SL)],
                    in_=kvbnc_out[i][ds(r * KVH, KH)].rearrange(
                        "(kc p tk) -> p kc tk", p=128, tk=SL))
        v_full = actp.tile([112, 6, H, HD + 1], BF16, name="v_full", tag="v_full")
        nc.vector.memset(v_full[:, :, :, HD:HD + 1], 1.0)
        for i in range(2):
            for c6 in range(6):
                off = (c6 // 3) * KVH + KH + (c6 % 3) * 112 * 512
                nc.gpsimd.dma_start(
                    out=v_full[:, c6, ds(i * 8, 8), 0:HD],
                    in_=kvbnc_out[i][ds(off, 112 * 512)].rearrange(
                        "(p hh e) -> p hh e", p=112, e=HD))

        # ---------------- attention ----------------
        boc = bias_sb[f"bo{l}"]
        att_sb = actp.tile([128, NKC, SL], BF16, name="att_sb", tag="att_sb")
        stage_o = actp.tile([64, NKC, SL], BF16, name="stage_o", tag="stage_o")

        for hc in range(8):
            esb2s = []
            for cc in range(6):
                tqs = SUF0[cc]
                suf = SL - tqs
                ps2 = ps_sc("pss")
                for par in range(2):
                    nc.tensor.matmul(
                        ps2[:, par, 0:suf],
                        lhsT=k_full[ds(64 * par, 64), hc, ts(cc, 112)],
                        rhs=q_sb[ds(64 * par, 64), hc, ds(tqs, suf)],
                        start=True, stop=True)
                esb2 = esbp.tile([112, 2, SL], BF16, name="esb2", tag="esb")
                nc.scalar.activation(out=esb2[:, :, 0:suf],
                                     in_=ps2[:, :, 0:suf], func=AF.Exp)
                for par in range(2):
                    nc.vector.tensor_mul(esb2[:, par, 0:suf],
                                         esb2[:, par, 0:suf],
                                         sb_mask[:, cc, ds(tqs, suf)])
                esb2s.append(esb2)
            for par in range(2):
                hh = 2 * hc + par
                psa = ps_sm([HD + 1, SL], "psa")
                for cc in range(6):
                    tqs = SUF0[cc]
                    suf = SL - tqs
                    nc.tensor.matmul(psa[:, ds(tqs, suf)],
                                     lhsT=v_full[:, cc, hh, :],
                                     rhs=esb2s[cc][:, par, 0:suf],
                                     start=(cc == 0), stop=(cc == 5))
                den = lnp.tile([1, SL], F32, name="den", tag="den", bufs=2)
                nc.vector.tensor_copy(out=den[:], in_=psa[ds(HD, 1), :])
                rec = lnp.tile([1, SL], F32, name="rec", tag="rec", bufs=2)
                nc.vector.reciprocal_approx_fast(out=rec[:], in_=den[:])
                rb = lnp.tile([64, SL], F32, name="rb", tag="rb", bufs=2)
                nc.gpsimd.partition_broadcast(out_ap=rb[:], in_ap=rec[:])
                dst = att_sb[0:64, hc, :] if par == 0 else stage_o[:, hc, :]
                nc.vector.tensor_mul(dst, psa[0:HD, :], rb[:])
            if hc == 3 or hc == 7:
                half = hc // 4
                nc.sync.dma_start(out=att_sb[ds(64, 64), ds(half * 4, 4), :],
                                  in_=stage_o[:, ds(half * 4, 4), :])
                for oc in range(NKC):
                    pso = ps_sm([128, SL], "pso")
                    for kc in range(4 * half, 4 * half + 4):
                        nc.tensor.matmul(pso[:], lhsT=wo[:, kc, ts(oc, 128)],
                                         rhs=att_sb[:, kc, :],
                                         start=(kc == 4 * half),
                                         stop=(kc == 4 * half + 3))
                    if half == 0:
                        nc.vector.scalar_tensor_tensor(
                            out=d1[:, oc, :], in0=pso[:],
                            scalar=boc[:, oc:oc + 1],
                            in1=h_f32[:, oc, :], op0=AX.add, op1=AX.add)
                    else:
                        nc.vector.tensor_add(d1[:, oc, :], d1[:, oc, :],
                                             pso[:])
        ln(bias_sb[f"ln1s{l}"], bias_sb[f"ln1b{l}"], src=d1)

        # FFN
        g_f8 = actp.tile([128, NFC, SL], F8, name="g_f8", tag="g_f8")
        b1c = bias_sb[f"b1{l}"]
        for quad in range(4):
            w1 = wpool.tile([128, NKP, 2048] if SWI else [128, NKC, 1024],
                            F8, name="w1q", tag="w1q")
            nc.scalar.dma_start(out=w1[:], in_=t[f"W1{l}"][quad])
            for f8c in range(8):
                fc = quad * 8 + f8c
                psf = ps_sm([128, SL], "psf")
                mm_dr(psf, w1, f8c, h_f8)
                nc.scalar.activation(out=g_f8[:, fc, :], in_=psf[:],
                                     func=AF.Gelu, scale=RW,
                                     bias=b1c[:, fc:fc + 1])
        b2c = bias_sb[f"b2{l}"]
        for oc in range(NKC):
            w2 = w2p.tile([128, NFC // 2, 256] if SWI else [128, NFC, 128],
                          F8, name="w2oc", tag="w2oc")
            nc.scalar.dma_start(out=w2[:], in_=t[f"W2{l}"][oc])
            psy = ps_sm([128, SL], "psy")
            mm_dr(psy, w2, 0, g_f8, nkp=NFC // 2)
            nc.vector.tensor_scalar(
                out=d1[:, oc, :], in0=psy[:], scalar1=b2c[:, oc:oc + 1],
                scalar2=RW, op0=AX.add, op1=AX.mult)
            nc.vector.tensor_add(d1[:, oc, :], d1[:, oc, :], h_f32[:, oc, :])

        ln(bias_sb[f"ln2s{l}"], bias_sb[f"ln2b{l}"], src=d1)

        if l + 1 < NL:
            wk = load_wsi(t[f"Wk{l + 1}"][:], "wk")
            wv = load_wv(l + 1)
            wq = load_wsi(t[f"Wq{l + 1}"][:], "wq")

    # ---------------- final: fused LN_f + pooling, head, MLP ----------------
    mean_bf, recf_bf, r1f = ln_stats(h_f32)
    mr1 = lnp.tile([1, SL], F32, name="mr1", tag="mr1")
    nc.vector.tensor_mul(mr1[:], mean_bf[:], r1f[:])
    csc = lnp.tile([1, 1], F32, name="csc", tag="csc")
    nc.vector.reduce_sum(out=csc[:], in_=mr1[:], axis=XL.X)
    rbf_ps = ps_sm([128, SL], "rbf_ps")
    nc.tensor.matmul(rbf_ps[:], lhsT=ones_row[:], rhs=recf_bf[:],
                     start=True, stop=True)
    hsum = constp.tile([128, NKC], F32, name="hsum", tag="hsum")
    for c8 in range(NKC):
        nc.vector.scalar_tensor_tensor(
            out=d1[:, c8, :], in0=h_f32[:, c8, :], scalar=1.0, in1=rbf_ps[:],
            op0=AX.mult, op1=AX.mult, accum_out=hsum[:, c8:c8 + 1])
    FD = D + 8
    fin_in = dramp.tile([FD], F32, name="fin_in", tag="fin_in")
    nc.sync.dma_start(out=fin_in[ds(0, D)].rearrange("(kc p) -> p kc", p=128),
                      in_=hsum[:])
    zpad = lnp.tile([1, 8], F32, name="zpad", tag="zpad")
    nc.vector.memset(zpad[:], 0.0)
    nc.sync.dma_start(out=fin_in[ds(D, 8)], in_=zpad[:])
    nc.sync.dma_start(out=fin_in[ds(D, 1)], in_=csc[:])
    fin_out = single([2 * FD], F32, "fin_out", space="DRAM",
                     addr_space="Shared")
    nc.gpsimd.collective_compute(
        "AllGather", AX.bypass, replica_groups=REPLICA_GROUPS,
        ins=[fin_in[:]], outs=[fin_out[:]])
    ffx = constp.tile([128, NKC, 2], F32, name="ffx", tag="ffx")
    for r in range(2):
        nc.sync.dma_start(
            out=ffx[:, :, r],
            in_=fin_out[ds(r * FD, D)].rearrange("(kc p) -> p kc", p=128))
    csc2 = constp.tile([1, 2], F32, name="csc2", tag="csc2")
    for r in range(2):
        nc.sync.dma_start(out=csc2[:, r:r + 1], in_=fin_out[ds(r * FD + D, 1)])
    cst_bf = constp.tile([1, 1], BF16, name="cst_bf", tag="cst_bf")
    nc.vector.tensor_add(cst_bf[:], csc2[:, 0:1], csc2[:, 1:2])
    hbar_bf = constp.tile([128, NKC], BF16, name="hbar_bf", tag="hbar_bf")
    nc.vector.tensor_add(hbar_bf[:], ffx[:, :, 0], ffx[:, :, 1])

    psh = ps_sm([OUT, 1], "psh")
    for kc in range(NKC):
        nc.tensor.matmul(psh[:], lhsT=sb_headW[:, kc, :],
                         rhs=hbar_bf[:, kc:kc + 1],
                         start=(kc == 0), stop=False)
    nc.tensor.matmul(psh[:], lhsT=sb_hwsumN[:], rhs=cst_bf[:],
                     start=False, stop=True)
    feat_bf = constp.tile([OUT, 1], BF16, name="feat_bf", tag="feat_bf")
    nc.scalar.activation(out=feat_bf[:], in_=psh[:], func=AF.Identity,
                         bias=sb_featC[:])

    z1_bf = constp.tile([128, 2], BF16, name="z1_bf", tag="z1_bf")
    for i2 in range(2):
        psc = ps_sm([128, 1], "psc")
        nc.tensor.matmul(psc[:], lhsT=sb_c1W[:, ts(i2, 128)], rhs=feat_bf[:],
                         start=True, stop=True)
        nc.scalar.activation(out=z1_bf[:, i2:i2 + 1], in_=psc[:], func=AF.Relu,
                             bias=sb_c1B[:, i2:i2 + 1])
    psc2 = ps_sm([64, 1], "psc2")
    for kc in range(2):
        nc.tensor.matmul(psc2[:], lhsT=sb_c2W[:, kc, :], rhs=z1_bf[:, kc:kc + 1],
                         start=(kc == 0), stop=(kc == 1))
    z2_bf = constp.tile([64, 1], BF16, name="z2_bf", tag="z2_bf")
    nc.scalar.activation(out=z2_bf[:], in_=psc2[:], func=AF.Relu, bias=sb_c2B[:])
    psc3 = ps_sm([2, 1], "psc3")
    nc.tensor.matmul(psc3[:], lhsT=sb_c3W[:], rhs=z2_bf[:], start=True, stop=True)
    out_sb = constp.tile([2, 1], F32, name="out_sb", tag="out_sb")
    nc.scalar.activation(out=out_sb[:], in_=psc3[:], func=AF.Identity,
                         bias=sb_c3B[:])
    nc.sync.dma_start(out=out_dram[:], in_=out_sb[:])
    ctx.close()


# ----------------------------------------------------------------------------
# host side
# ----------------------------------------------------------------------------

def _bf16(x):
    return np.ascontiguousarray(np.asarray(x, dtype=np.float32)).astype(
        ml_dtypes.bfloat16)


def _f32(x):
    return np.ascontiguousarray(np.asarray(x, dtype=np.float32))


def _f8(x):
    return np.ascontiguousarray(
        np.clip(np.asarray(x, dtype=np.float32), -240.0, 240.0)).astype(
        ml_dtypes.float8_e4m3)


def _w8i(a, scale=WS):
    """[D_in, N] -> SwInterleave strips [128, NKP, 2N]: per k-pair, columns
    of a 128-col chunk reversed + A/B interleaved, chunks concatenated."""
    a = np.asarray(a, np.float32) * scale
    din, n = a.shape
    kc = din // 128
    ar = a.reshape(kc, 128, n // 128, 128)
    A = ar[0::2][..., ::-1]                     # [kc/2, 128, nc, 128] reversed
    Bm = ar[1::2][..., ::-1]
    inter = np.stack([A, Bm], axis=-1)          # [kc/2, 128, nc, 128, 2]
    inter = inter.transpose(1, 0, 2, 3, 4).reshape(128, kc // 2, 2 * n)
    return _f8(inter)


def _w8p(a, scale=WS):
    """[D_in, N] -> plain [128, D_in//128, N] fp8 (p, kc, n)."""
    a = np.asarray(a, np.float32) * scale
    din, n = a.shape
    return _f8(a.reshape(din // 128, 128, n).transpose(1, 0, 2))


def _wtile(a):
    a = np.asarray(a, np.float32)
    din, n = a.shape
    return _bf16(a.reshape(din // 128, 128, n).transpose(1, 0, 2))


def _w2comp(inp, l):
    """Static mean-compensation for fp8 W2: (W2 - W2q)^T @ E[gelu] where the
    gelu input is ~N(0, sigma_j) with sigma_j = ||diag(ln1_s) W1[:, j]||."""
    w2 = np.asarray(inp["W2"][l], np.float32)
    w2q = np.clip(w2 * WS, -240, 240).astype(ml_dtypes.float8_e4m3).astype(
        np.float32) / WS
    w1 = np.asarray(inp["W1"][l], np.float32)
    s = np.asarray(inp["ln1_s"][l], np.float32)
    sig = np.linalg.norm(w1 * s[:, None], axis=0)
    ebar = sig ** 2 / np.sqrt(2 * np.pi * (1 + sig ** 2))
    return (w2 - w2q).T @ ebar


def _btile(a, p=128):
    a = np.asarray(a, np.float32)
    return _f32(a.reshape(-1, p).T)


def _host_weights(inp):
    w = {}
    w["embW"] = _bf16(inp["emb_W"])
    bias_cols = [_btile(-np.asarray(inp["emb_W"], np.float32).sum(0)),
                 _btile(inp["emb_b"])]
    for l in range(NL):
        _wst = _w8i if SWI else _w8p
        w[f"Wq{l}"] = _wst(np.asarray(inp["Wq"][l], np.float32) * 0.125)
        w[f"Wk{l}"] = _wst(inp["Wk"][l])
        w[f"Wo{l}"] = _wtile(inp["Wo"][l])
        wv = _w8p(inp["Wv"][l])                  # [128, 8, 1024]
        w[f"Wv{l}"] = np.ascontiguousarray(
            wv.reshape(128, NKC, 2, 512).transpose(2, 0, 1, 3))
        w1 = np.asarray(inp["W1"][l], np.float32)
        if SWI:
            w1i = _w8i(w1)                       # [128, 4, 8192]
            w[f"W1{l}"] = np.ascontiguousarray(
                w1i.reshape(128, NKP, 4, 2048).transpose(2, 0, 1, 3))
        else:
            w1p = _w8p(w1)                       # [128, 8, 4096]
            w[f"W1{l}"] = np.ascontiguousarray(
                w1p.reshape(128, NKC, 4, 1024).transpose(2, 0, 1, 3))
        w2 = np.asarray(inp["W2"][l], np.float32)
        # per-oc SwI: treat each oc as [DFF, 128] -> [128part, NFC/2, 256]
        w2oc = []
        for oc in range(NKC):
            w2oc.append(_w8i(w2[:, ds_np(oc)]) if SWI
                        else _w8p(w2[:, ds_np(oc)]))
        w[f"W2{l}"] = np.ascontiguousarray(np.stack(w2oc))
        bias_cols += [
            _btile(np.asarray(inp["bq"][l], np.float32) * 8.0),
            _btile(np.asarray(inp["bk"][l], np.float32) * WS),
            _btile(inp["bo"][l]),
            _btile((np.asarray(inp["b2"][l], np.float32) + _w2comp(inp, l)) * WS),
            _btile(inp["ln1_s"][l]),
            _btile(inp["ln1_b"][l]),
            _btile(inp["ln2_s"][l]),
            _btile(inp["ln2_b"][l]),
            _btile(inp["b1"][l]),
        ]
    w["biases"] = _f32(np.concatenate(bias_cols, axis=1))
    w["bvpack"] = _bf16(np.concatenate(
        [np.asarray(inp["bv"][l], np.float32) for l in range(NL)])[None, :])
    lnfs = np.asarray(inp["lnf_s"], np.float32)
    lnfb = np.asarray(inp["lnf_b"], np.float32)
    hW = np.asarray(inp["head_W"], np.float32)
    hws = hW * lnfs[:, None] / S
    w["headWs"] = _wtile(hws)
    w["hwsumN"] = _bf16(-hws.sum(0)[None, :])
    w["featC"] = _f32((hW.T @ lnfb + np.asarray(inp["head_b"], np.float32))[:, None])
    w["c1W"] = _bf16(inp["c1_W"])
    w["c1B"] = _btile(inp["c1_b"])
    w["c2W"] = _wtile(inp["c2_W"])
    w["c2B"] = _f32(np.asarray(inp["c2_b"], np.float32)[:, None])
    w["c3W"] = _bf16(inp["c3_W"])
    w["c3B"] = _f32(np.asarray(inp["c3_b"], np.float32)[:, None])
    return w


def ds_np(oc):
    return slice(oc * 128, (oc + 1) * 128)


def kernel(**inputs):
    global _BUILT, LAST_RESULT
    if _BUILT is None:
        _BUILT = _build()
    nc, names = _BUILT

    w = _host_weights(inputs)
    x = np.asarray(inputs["x"], np.float32)

    wk = np.concatenate([np.repeat(np.arange(16) * 2, C),
                         np.repeat(np.arange(16) * 2 + 1, C)])
    in_maps = []
    for core in range(8):
        b, parity = core // 2, core % 2
        wins = np.arange(16) * 2 + parity
        xb = x[b]
        xl = np.empty((P, SL), np.float32)
        for i, wn in enumerate(wins):
            xl[:, i * C:(i + 1) * C] = xb[wn * P:(wn + 1) * P, :]
        wq = np.repeat(wins, C)
        mask = (wk[:, None] <= wq[None, :]).astype(np.float32)
        mask3 = mask.reshape(6, 112, SL).transpose(1, 0, 2)
        m = dict(w)
        m["xfull"] = _f32(xb.T)
        m["xloc"] = _bf16(xl)
        m["maskM"] = _f8(mask3)
        in_maps.append(m)

    res = run_bass_kernel_spmd(nc, in_maps, core_ids=list(range(8)))
    LAST_RESULT = res
    logits = np.stack(
        [res.results[2 * b]["out"].reshape(2).astype(np.float32) for b in range(B)])
    return logits
